# revision 6
# baseline (speedup 1.0000x reference)
"""Trainium2 Bass kernel for nn_PredictionLoss (embedding-lookup dot + 12*tanh + SmoothL1).

Math (reference):
    scores[b] = input[b] . centers[label[b]]        # only the labeled row is needed
    predict_a[b] = 12 * tanh(scores[b])
    loss = mean(smooth_l1(predict_a - factor))      # beta=1

Sharding: data-parallel over the batch B=65536 across 8 cores (8192 rows each).
centers (1299, 512) stays in HBM; each core gathers its needed rows with
dma_gather (SWDGE indirect DMA, 2KB/row descriptors at line rate).

Per-core layout trick: dma_gather writes gathered row i to SBUF partition i%128,
column-block i//128.  The input DMA uses a matching strided access pattern so the
two tiles line up elementwise; the dot is then a per-partition fused
multiply-reduce (tensor_tensor_reduce) over the 512-wide free dim.

Loss: per-element l = m*(ad - 0.5*m) with ad=|d|, m=min(ad,1)  (branch-free
SmoothL1).  Expanded as  sum(m*ad) - 0.5*sum(m^2)  so both terms come out of
tensor_tensor_reduce accumulators for free.  Per-core partial sums [128,1] are
summed on host (the all-reduce of the mean).
"""

import numpy as np

B = 65536
F = 512
K = 1299
N_CORES = 8
ROWS_PER_CORE = B // N_CORES          # 8192
ROWS_PER_CHUNK = 1024                 # one dma_gather per chunk
N_CHUNKS = ROWS_PER_CORE // ROWS_PER_CHUNK   # 8
CB = ROWS_PER_CHUNK // 128            # column blocks per chunk = 8
N_COLS = ROWS_PER_CORE // 128         # 64 columns in the (128, N_COLS) row layout

_compiled = None


def _build():
    from concourse import bacc, mybir, tile

    nc = bacc.Bacc("TRN2", target_bir_lowering=False, debug=False)
    f32 = mybir.dt.float32
    i16 = mybir.dt.int16

    x = nc.dram_tensor("x", [ROWS_PER_CORE, F], f32, kind="ExternalInput")
    centers = nc.dram_tensor("centers", [K, F], f32, kind="ExternalInput")
    fac = nc.dram_tensor("fac", [128, N_COLS], f32, kind="ExternalInput")
    idxs = nc.dram_tensor("idxs", [128, ROWS_PER_CORE // 16], i16, kind="ExternalInput")
    pred = nc.dram_tensor("pred", [128, N_COLS], f32, kind="ExternalOutput")
    lpart = nc.dram_tensor("lpart", [128, 1], f32, kind="ExternalOutput")

    AT = mybir.AluOpType
    ACT = mybir.ActivationFunctionType

    with tile.TileContext(nc) as tc:
        with tc.tile_pool(name="big", bufs=3) as big, \
             tc.tile_pool(name="small", bufs=2) as small, \
             tc.tile_pool(name="persist", bufs=1) as persist:

            idx_sb = persist.tile([128, ROWS_PER_CORE // 16], i16)
            nc.sync.dma_start(idx_sb[:], idxs[:])
            fac_sb = persist.tile([128, N_COLS], f32)
            nc.sync.dma_start(fac_sb[:], fac[:])

            pa_sb = persist.tile([128, N_COLS], f32)
            lbuf = persist.tile([128, N_COLS], f32)   # per-element smoothl1

            for c in range(N_CHUNKS):
                r0 = c * ROWS_PER_CHUNK
                # input rows r0+j*128+p  ->  xb[p, j, :]
                xb = big.tile([128, CB, F], f32, tag="xb")
                nc.sync.dma_start(
                    xb[:],
                    x[r0:r0 + ROWS_PER_CHUNK, :].rearrange("(j p) f -> p j f", p=128),
                )
                # centers[label[r0+i]] -> cg[i%128, i//128, :]
                cg = big.tile([128, CB, F], f32, tag="cg")
                nc.gpsimd.dma_gather(
                    cg[:],
                    centers[:, :],
                    idx_sb[:, c * (ROWS_PER_CHUNK // 16):(c + 1) * (ROWS_PER_CHUNK // 16)],
                    ROWS_PER_CHUNK,
                    ROWS_PER_CHUNK,
                    F,
                )

                prod = big.tile([128, CB, F], f32, tag="prod")
                nc.vector.tensor_tensor(
                    out=prod[:], in0=xb[:], in1=cg[:], op=AT.mult
                )
                dot = small.tile([128, CB], f32, tag="dot")
                nc.vector.tensor_reduce(
                    dot[:], prod[:], mybir.AxisListType.X, AT.add
                )

                sl = slice(c * CB, (c + 1) * CB)
                t = small.tile([128, CB], f32, tag="t")
                nc.scalar.activation(t[:], dot[:], ACT.Tanh)
                # predict_a = 12 * tanh(dot)
                nc.vector.tensor_scalar(
                    out=pa_sb[:, sl], in0=t[:], scalar1=12.0, scalar2=None, op0=AT.mult
                )
                d = small.tile([128, CB], f32, tag="d")
                nc.vector.tensor_tensor(
                    out=d[:], in0=pa_sb[:, sl], in1=fac_sb[:, sl], op=AT.subtract
                )
                ad = small.tile([128, CB], f32, tag="ad")
                nc.scalar.activation(ad[:], d[:], ACT.Abs)
                m = small.tile([128, CB], f32, tag="m")
                nc.vector.tensor_scalar(
                    out=m[:], in0=ad[:], scalar1=1.0, scalar2=None, op0=AT.min
                )
                # smoothl1 = m*(ad - 0.5*m), branch-free (m = min(|d|, 1))
                hm = small.tile([128, CB], f32, tag="hm")
                nc.vector.tensor_scalar(
                    out=hm[:], in0=m[:], scalar1=-0.5, scalar2=None, op0=AT.mult
                )
                s = small.tile([128, CB], f32, tag="s")
                nc.vector.tensor_tensor(out=s[:], in0=ad[:], in1=hm[:], op=AT.add)
                nc.vector.tensor_tensor(
                    out=lbuf[:, sl], in0=m[:], in1=s[:], op=AT.mult
                )

            nc.sync.dma_start(pred[:], pa_sb[:])
            lsum = small.tile([128, 1], f32, tag="lsum")
            nc.vector.tensor_reduce(lsum[:], lbuf[:], mybir.AxisListType.X, AT.add)
            nc.sync.dma_start(lpart[:], lsum[:])

    nc.compile()
    return nc


def _get_compiled():
    global _compiled
    if _compiled is None:
        _compiled = _build()
    return _compiled


def _make_in_maps(input, factor, label, centers):
    centers2d = np.ascontiguousarray(centers.reshape(K, F).astype(np.float32))
    lab16 = label.astype(np.int16)
    in_maps = []
    for core in range(N_CORES):
        s0 = core * ROWS_PER_CORE
        xs = np.ascontiguousarray(input[s0:s0 + ROWS_PER_CORE].astype(np.float32))
        fshard = factor[s0:s0 + ROWS_PER_CORE].reshape(N_COLS, 128).T
        fshard = np.ascontiguousarray(fshard.astype(np.float32))
        lshard = lab16[s0:s0 + ROWS_PER_CORE]
        # dma_gather's Q7 core pair reads the [16, n/16] index block from its
        # own 16-partition group -> replicate the block across all 128.
        idx = np.zeros((128, ROWS_PER_CORE // 16), dtype=np.int16)
        for c in range(N_CHUNKS):
            w = ROWS_PER_CHUNK // 16  # 64
            blk = lshard[c * ROWS_PER_CHUNK:(c + 1) * ROWS_PER_CHUNK].reshape(w, 16).T
            idx[:, c * w:(c + 1) * w] = np.tile(blk, (8, 1))
        in_maps.append({"x": xs, "centers": centers2d, "fac": fshard, "idxs": idx})
    return in_maps


def kernel(input, factor, label, centers, _trace=False):
    from concourse import bass_utils

    nc = _get_compiled()
    in_maps = _make_in_maps(
        np.asarray(input), np.asarray(factor), np.asarray(label), np.asarray(centers)
    )
    res = bass_utils.run_bass_kernel_spmd(
        nc, in_maps, core_ids=list(range(N_CORES)), trace=_trace
    )
    outs = res.results
    pa = np.concatenate(
        [outs[core]["pred"].T.reshape(-1) for core in range(N_CORES)]
    ).reshape(B, 1).astype(np.float32)
    total = sum(float(outs[core]["lpart"].astype(np.float64).sum())
                for core in range(N_CORES))
    loss = np.float32(total / B)
    if _trace:
        return (loss, pa), res
    return (loss, pa)


# revision 8
# speedup vs baseline: 1.0077x; 1.0077x over previous
"""Trainium2 Bass kernel for nn_PredictionLoss (embedding-lookup dot + 12*tanh + SmoothL1).

Math (reference):
    scores[b] = input[b] . centers[label[b]]        # only the labeled row is needed
    predict_a[b] = 12 * tanh(scores[b])
    loss = mean(smooth_l1(predict_a - factor))      # beta=1

Sharding: data-parallel over the batch B=65536 across 8 cores (8192 rows each).
centers (1299, 512) stays in HBM; each core gathers its needed rows with
dma_gather (SWDGE indirect DMA, 2KB/row descriptors), spread over all 4 SWDGE
queues so descriptor generation runs on all 4 Q7 core pairs in parallel.

Row->position mapping (free host-side choice, baked into the gather indices,
factor layout and output unshard): position [p, c] with c = chunk*8 + j holds
shard row  chunk*1024 + p*8 + j.  This makes the x load per-partition
CONTIGUOUS (128 descriptors x 16KB per 2MB chunk - line rate), while the
gather's fixed i -> [i%128, i//128] layout is absorbed by permuting the index
array on host.

Dot: in-place elementwise mult (DVE) + per-chunk reduce; reduces alternate
between DVE tensor_reduce and ACT activation(Copy, accum_out) to balance
engine load under the DMA roofline.

Loss: per-element smoothl1 via the branch-free identity l = m*(ad - 0.5m),
m = min(ad,1); computed once on [128,64] tiles at the end; device returns
lm = -0.5*l sums per partition, host scales by -2 and divides by B.
"""

import numpy as np

B = 65536
F = 512
K = 1299
N_CORES = 8
ROWS_PER_CORE = B // N_CORES          # 8192
ROWS_PER_CHUNK = 1024                 # one dma_gather per chunk
N_CHUNKS = ROWS_PER_CORE // ROWS_PER_CHUNK   # 8
CB = ROWS_PER_CHUNK // 128            # column blocks per chunk = 8
N_COLS = ROWS_PER_CORE // 128         # 64 columns in the (128, N_COLS) layout

_compiled = None


def _build():
    from concourse import bacc, mybir, tile

    nc = bacc.Bacc("TRN2", target_bir_lowering=False, debug=False,
                   num_swdge_queues=4)
    f32 = mybir.dt.float32
    i16 = mybir.dt.int16

    x = nc.dram_tensor("x", [ROWS_PER_CORE, F], f32, kind="ExternalInput")
    centers = nc.dram_tensor("centers", [K, F], f32, kind="ExternalInput")
    fac = nc.dram_tensor("fac", [128, N_COLS], f32, kind="ExternalInput")
    idxs = nc.dram_tensor("idxs", [128, ROWS_PER_CORE // 16], i16, kind="ExternalInput")
    pred = nc.dram_tensor("pred", [128, N_COLS], f32, kind="ExternalOutput")
    lpart = nc.dram_tensor("lpart", [128, 1], f32, kind="ExternalOutput")

    AT = mybir.AluOpType
    ACT = mybir.ActivationFunctionType

    with tile.TileContext(nc) as tc:
        with tc.tile_pool(name="big", bufs=3) as big, \
             tc.tile_pool(name="small", bufs=2) as small, \
             tc.tile_pool(name="persist", bufs=1) as persist:

            idx_sb = persist.tile([128, ROWS_PER_CORE // 16], i16)
            nc.sync.dma_start(idx_sb[:], idxs[:])
            fac_sb = persist.tile([128, N_COLS], f32)
            nc.sync.dma_start(fac_sb[:], fac[:])

            dot_sb = persist.tile([128, N_COLS], f32)
            pa_sb = persist.tile([128, N_COLS], f32)

            for c in range(N_CHUNKS):
                r0 = c * ROWS_PER_CHUNK
                # x rows r0 + p*8 + j  ->  xb[p, j, :]   (16KB contiguous/partition)
                xb = big.tile([128, CB, F], f32, tag="xb")
                nc.sync.dma_start(
                    xb[:],
                    x[r0:r0 + ROWS_PER_CHUNK, :].rearrange("(p j) f -> p j f", p=128),
                )
                # centers[idx[i]] -> cg[i%128, i//128, :]; host permuted idx so
                # that cg[p, j] == centers[label[row at (p, c*8+j)]]
                cg = big.tile([128, CB, F], f32, tag="cg")
                nc.gpsimd.dma_gather(
                    cg[:],
                    centers[:, :],
                    idx_sb[:, c * (ROWS_PER_CHUNK // 16):(c + 1) * (ROWS_PER_CHUNK // 16)],
                    ROWS_PER_CHUNK,
                    ROWS_PER_CHUNK,
                    F,
                    queue_num=c % 4,
                )

                # in-place: cg *= xb
                nc.vector.tensor_mul(cg[:], xb[:], cg[:])
                sl = slice(c * CB, (c + 1) * CB)
                if c % 2 == 0:
                    nc.vector.tensor_reduce(
                        dot_sb[:, sl], cg[:], mybir.AxisListType.X, AT.add
                    )
                else:
                    # ACT-side reduce: Copy with accumulate, one call per block
                    for j in range(CB):
                        scr = small.tile([128, F], f32, tag="scr")
                        nc.scalar.activation(
                            scr[:], cg[:, j, :], ACT.Copy,
                            accum_out=dot_sb[:, c * CB + j:c * CB + j + 1],
                        )

            # ---- tail: predict_a + smoothl1 on [128, 64] tiles ----
            t = small.tile([128, N_COLS], f32, tag="t")
            nc.scalar.activation(t[:], dot_sb[:], ACT.Tanh)
            nc.vector.tensor_scalar(
                out=pa_sb[:], in0=t[:], scalar1=12.0, scalar2=None, op0=AT.mult
            )
            nc.sync.dma_start(pred[:], pa_sb[:])

            d = small.tile([128, N_COLS], f32, tag="d")
            nc.vector.tensor_tensor(out=d[:], in0=pa_sb[:], in1=fac_sb[:],
                                    op=AT.subtract)
            ad = small.tile([128, N_COLS], f32, tag="ad")
            nc.scalar.activation(ad[:], d[:], ACT.Abs)
            # hm = -0.5 * min(ad, 1)
            hm = small.tile([128, N_COLS], f32, tag="hm")
            nc.vector.tensor_scalar(
                out=hm[:], in0=ad[:], scalar1=1.0, scalar2=-0.5,
                op0=AT.min, op1=AT.mult,
            )
            # s = ad + hm = ad - 0.5*m ;  lm = hm*s = -0.5 * smoothl1
            s = small.tile([128, N_COLS], f32, tag="s")
            nc.vector.tensor_tensor(out=s[:], in0=ad[:], in1=hm[:], op=AT.add)
            lm = small.tile([128, N_COLS], f32, tag="lm")
            nc.vector.tensor_tensor(out=lm[:], in0=hm[:], in1=s[:], op=AT.mult)
            lsum = small.tile([128, 1], f32, tag="lsum")
            nc.vector.tensor_reduce(lsum[:], lm[:], mybir.AxisListType.X, AT.add)
            nc.sync.dma_start(lpart[:], lsum[:])

    nc.compile()
    return nc


def _get_compiled():
    global _compiled
    if _compiled is None:
        _compiled = _build()
    return _compiled


def _make_in_maps(input, factor, label, centers):
    centers2d = np.ascontiguousarray(centers.reshape(K, F).astype(np.float32))
    lab_all = np.asarray(label)
    in_maps = []
    for core in range(N_CORES):
        s0 = core * ROWS_PER_CORE
        xs = np.ascontiguousarray(np.asarray(input)[s0:s0 + ROWS_PER_CORE],
                                  dtype=np.float32)
        fshard = np.asarray(factor)[s0:s0 + ROWS_PER_CORE].reshape(
            N_CHUNKS, 128, CB)                      # [chunk, p, j]
        fac_sb = np.ascontiguousarray(
            fshard.transpose(1, 0, 2).reshape(128, N_COLS), dtype=np.float32)
        lshard = lab_all[s0:s0 + ROWS_PER_CORE].astype(np.int16).reshape(
            N_CHUNKS, 128, CB)                      # [chunk, p, j]
        # gather idx i (of chunk) sits at [i%128, i//128] == (p, j)
        # -> idx_chunk[i = j*128 + p] = label[row at (p, j)] = lshard[c, p, j]
        idx = np.empty((128, ROWS_PER_CORE // 16), dtype=np.int16)
        w = ROWS_PER_CHUNK // 16  # 64
        for c in range(N_CHUNKS):
            flat = lshard[c].T.reshape(-1)          # i = j*128 + p order
            blk = flat.reshape(w, 16).T             # [16, 64] wrap (p=i%16, s=i//16)
            idx[:, c * w:(c + 1) * w] = np.tile(blk, (8, 1))
        in_maps.append({"x": xs, "centers": centers2d, "fac": fac_sb, "idxs": idx})
    return in_maps


def kernel(input, factor, label, centers, _trace=False):
    import os

    from concourse import bass_utils

    if not _trace:
        # the slim axon client lacks the NTFF hook module; make sure an
        # inherited BASS_TRACE can't push us onto that path
        os.environ["BASS_NEVER_TRACE"] = "1"
    else:
        os.environ.pop("BASS_NEVER_TRACE", None)

    nc = _get_compiled()
    in_maps = _make_in_maps(input, factor, label, centers)
    res = bass_utils.run_bass_kernel_spmd(
        nc, in_maps, core_ids=list(range(N_CORES)), trace=_trace
    )
    outs = res.results
    pa = np.empty((B,), dtype=np.float32)
    total = 0.0
    for core in range(N_CORES):
        pred = outs[core]["pred"]                   # [128, 64], [p, chunk*8+j]
        # row chunk*1024 + p*8 + j  <-  pred[p, chunk*8 + j]
        shard = pred.reshape(128, N_CHUNKS, CB).transpose(1, 0, 2).reshape(-1)
        pa[core * ROWS_PER_CORE:(core + 1) * ROWS_PER_CORE] = shard
        total += float(outs[core]["lpart"].astype(np.float64).sum())
    loss = np.float32(total * (-2.0) / B)
    pa = pa.reshape(B, 1)
    if _trace:
        return (loss, pa), res
    return (loss, pa)
